# revision 60
# baseline (speedup 1.0000x reference)
"""Trainium2 Bass kernel for the GCNN message-passing module.

Strategy (8-way data/graph parallel, nodes sharded by destination):
  - Each core owns a contiguous block of 2560 destination nodes (N padded
    20000 -> 20480). Relation weights are replicated to every core's HBM.
  - Messages (2 per edge: head<-W_r(tail), tail<-W_{r+R}(head)) are
    partitioned by destination on the host, sorted by (relation, dest),
    and packed into 128-message chunks per (relation, 512-dest window),
    with chunk boundaries at uniform dest values (so the one-hot matmul
    windows tile [0,512) with minimal cross-core widening).
  - The per-message source rows are laid out host-side (pure indexing)
    into a chunk-ordered DRAM tensor xg, so the device streams them with
    plain contiguous DMAs -- no SWDGE descriptor generation on gpsimd.
  - On device, per (relation, window): stream the chunk block, then one
    PE matmul per (chunk, feature-block) with a one-hot assignment matrix
    A[msg, dest] built on-chip: G^T[feat, dest] += X_chunk^T @ A -- the PE
    does the transpose AND the segment-sum in one pass, accumulating in
    PSUM. Evacuation to SBUF alternates Act/DVE engines.
  - Dense transform straight to [dest, feat]: per 128-dest block,
    out = cnt @ b_aug + sum_w G_w^T-block @ W_w^T + x-block @ W_self^T,
    with G/x/cnt blocks as the stationary operand -- all bf16 matmuls
    accumulating in one f32 PSUM bank per block (self closes each chain
    so ipt/wt16 stay off the startup critical path). ReLU on DVE,
    bf16 store, DMA out.
  - Host concatenates the 8 disjoint row shards (output rows per core
    are a contiguous node range; no collectives anywhere).
"""

import os
import sys
from dataclasses import dataclass

sys.path.insert(0, "/opt/trn_rl_repo")

import numpy as np
import ml_dtypes

import concourse.bass as bass
import concourse.bacc as bacc
import concourse.tile as tile
from concourse import bass_utils, mybir
from concourse.bass import ds, ts
BF16 = ml_dtypes.bfloat16
NCORES = 8
R = 8
NW = 2 * R          # 16 relation weights
D = 512
KB = D // 128       # feature blocks
P = 128
SW = 512            # dest super-window (one PSUM bank of f32)

LAST_RESULT = None  # BassKernelResults of the last kernel() call (for test.py)


@dataclass
class Cfg:
    N: int          # true number of nodes
    NPAD: int       # padded to NCORES * NSW * SW
    CORE_NODES: int
    NSW: int


def make_cfg(n_nodes: int) -> Cfg:
    per_core = -(-n_nodes // NCORES)
    nsw = -(-per_core // SW)
    core_nodes = nsw * SW
    return Cfg(N=n_nodes, NPAD=core_nodes * NCORES, CORE_NODES=core_nodes, NSW=nsw)


def _host_prep(cfg, inp, heads, tails, rel, W_self, b_self, W_rel, b_rel):
    """Build per-core input tensors + the (uniform across cores) chunk plan."""
    NSW, CORE_NODES = cfg.NSW, cfg.CORE_NODES
    NKEY = NW * NSW

    dest = np.concatenate([heads, tails]).astype(np.int64)
    srcs = np.concatenate([tails, heads]).astype(np.int64)
    wgt = np.concatenate([rel, rel + R]).astype(np.int64)

    core = dest // CORE_NODES
    percore = []
    for c in range(NCORES):
        m = core == c
        dl = dest[m] - c * CORE_NODES
        s = srcs[m]
        w = wgt[m]
        order = np.lexsort((dl, w))
        dl, s, w = dl[order], s[order], w[order]
        key = w * NSW + dl // SW
        cnts = np.bincount(key, minlength=NKEY)
        koff = np.concatenate([[0], np.cumsum(cnts)])
        percore.append((dl, s, key, koff))

    allcnts = np.stack([np.diff(p[3]) for p in percore])         # [8, NKEY]
    nsl = np.maximum(-(-allcnts.max(0) // P), 1).astype(np.int64)

    # chunk plan: per key, start from uniform dest windows and recursively
    # split any window whose max-core message count exceeds 128, so every
    # chunk holds <=128 messages per core AND the windows tile [0, SW)
    # exactly once -- the one-hot agg matmuls stream each PSUM column
    # exactly one time per key. Only chunk 0 needs start=True.
    plan = []
    key_meta = {}    # per key: (cb, [(lo, hi)] leaves in dest order)
    cb = 0
    # sw-major chunk ordering: sw0's chunks (and dr columns) are contiguous
    # at the front, so the startup loads only a small dr slice
    for k in sorted(range(NKEY), key=lambda kk: (kk % NSW, kk // NSW)):
        arrs = []
        for c in range(NCORES):
            dl, s, key, koff = percore[c]
            arrs.append(dl[koff[k] : koff[k + 1]] % SW)  # sorted within key

        def maxcnt(lo, hi):
            return max(
                int(np.searchsorted(a, hi) - np.searchsorted(a, lo))
                for a in arrs
            )

        n = int(nsl[k])
        leaves = []

        def split(lo, hi):
            if hi - lo > 1 and maxcnt(lo, hi) > P:
                mid = (lo + hi) // 2
                split(lo, mid)
                split(mid, hi)
            else:
                leaves.append((lo, hi))

        for j in range(n):
            split(round(SW * j / n), round(SW * (j + 1) / n))
        wins = [(lo, hi - lo) for lo, hi in leaves]
        key_meta[k] = (cb, leaves)
        plan.append((k // NSW, k % NSW, cb, len(wins), wins))
        cb += len(wins)
    NCHUNK = cb
    NCHMAX = max(n for _, _, _, n, _ in plan)

    inp_bf = np.zeros((cfg.NPAD, D), BF16)
    inp_bf[: cfg.N] = inp.astype(BF16)

    # per-core: fill xg (pre-gathered source rows, chunk-ordered) and dr
    xgs, drs = [], []
    cnt_alls = [None] * NCORES
    for c in range(NCORES):
        dl, s, key, koff = percore[c]
        col = np.zeros(len(dl), np.int64)
        row = np.zeros(len(dl), np.int64)
        dr_rebase = np.zeros(len(dl), np.float64)
        for k in range(NKEY):
            cb_k, leaves = key_meta[k]
            arr = dl[koff[k] : koff[k + 1]] % SW
            base = koff[k]
            for j, (lo, hi) in enumerate(leaves):
                a = base + np.searchsorted(arr, lo)
                b = base + np.searchsorted(arr, hi)
                col[a:b] = cb_k + j
                row[a:b] = np.arange(b - a)
                dr_rebase[a:b] = dl[a:b] % SW - lo
        xg = np.zeros((P, NCHUNK, D), BF16)
        xg[row, col] = inp_bf[s]
        dr = np.full((P, NCHUNK), -1.0, np.float32)
        dr[row, col] = dr_rebase
        xgs.append(xg.reshape(P, NCHUNK * D))
        drs.append(np.ascontiguousarray(dr))

    # message counts per (w, dest) for the bias term
    for c in range(NCORES):
        m = core == c
        dlc = dest[m] - c * CORE_NODES
        wc = wgt[m]
        cnt_all = np.zeros((32, CORE_NODES), np.float32)
        np.add.at(cnt_all, (wc, dlc), 1.0)
        cnt_all[16, :] = 1.0
        cnt_alls[c] = cnt_all

    # W^T packed [p, 17, kb, o]: slice [:, w, kb, ob*128:(ob+1)*128] is the
    # [K=feat-block, M=out-block] stationary operand.
    Wall = np.concatenate([W_rel, W_self[None]], 0)              # [17, o, in]
    wt = np.ascontiguousarray(
        Wall.transpose(2, 0, 1).reshape(KB, P, 17, D).transpose(1, 2, 0, 3)
    ).astype(BF16)                                               # [p,17,kb,o]

    baug = np.zeros((32, D), np.float32)
    baug[:NW] = b_rel
    baug[16] = b_self
    baug = baug.astype(BF16)

    iota = np.tile(np.arange(SW, dtype=np.float16), (P, 1))

    in_maps = []
    for c in range(NCORES):
        sl = inp_bf[c * CORE_NODES : (c + 1) * CORE_NODES].astype(np.float32)
        ipt = np.ascontiguousarray(
            sl.T.reshape(KB, P, CORE_NODES).transpose(1, 0, 2)
        ).astype(BF16)                                           # [p, kb, j]
        in_maps.append(
            {
                "xg": xgs[c],
                "dr": drs[c],
                "iota": iota,
                "ipt": ipt,
                "wt": wt,
                "cnt": cnt_alls[c].astype(BF16),
                "baug": baug,
            }
        )
    return in_maps, plan, NCHUNK, NCHMAX


def _emit(tc, out_ap, ins, cfg, plan, NCHUNK, NCHMAX):
    nc = tc.nc
    f32 = mybir.dt.float32
    bf16 = mybir.dt.bfloat16
    NSW, CORE_NODES = cfg.NSW, cfg.CORE_NODES

    # plan indexed by (w, sw)
    bykey = {}
    for w, sw, cb, n, wins in plan:
        bykey[(w, sw)] = (cb, n, wins)

    with (
        tc.tile_pool(name="const", bufs=1) as const,
        tc.tile_pool(name="xp", bufs=7) as xp,
        tc.tile_pool(name="apl", bufs=14) as apl,
        tc.tile_pool(name="gsb", bufs=6) as gsb,
        tc.tile_pool(name="osb", bufs=6) as osb,
        tc.tile_pool(name="gps", bufs=4, space="PSUM") as gps,
        tc.tile_pool(name="aps", bufs=4, space="PSUM") as aps,
    ):
        # small consts first on the SP queue so the agg path starts
        # immediately; ipt on the Act queue in parallel; wt sliced per-w
        # (the dense-agg step consumes w in order, so w=0 lands early).
        xg_dram = ins["xg"]
        sw0_hi = max(cb + n for w, sw, cb, n, wins in plan if sw == 0)
        dr_sb = const.tile([P, NCHUNK], f32)
        nc.sync.dma_start(dr_sb[:, :sw0_hi], ins["dr"][:, :sw0_hi])
        iota_sb = const.tile([P, SW], mybir.dt.float16)
        nc.sync.dma_start(iota_sb[:], ins["iota"][:, :])
        cnt_sb = const.tile([32, CORE_NODES], bf16)
        nc.sync.dma_start(cnt_sb[:], ins["cnt"][:, :])
        baug_sb = const.tile([32, D], bf16)
        nc.sync.dma_start(baug_sb[:], ins["baug"][:, :])

        # prefetch the first keys' message blocks on the Pool queue
        xts = {}
        for w0 in range(2):
            cb, n, _ = bykey[(w0, 0)]
            xt = xp.tile([P, NCHMAX * D], bf16, tag="x")
            nc.gpsimd.dma_start(xt[:, : n * D], xg_dram[:, cb * D : (cb + n) * D])
            xts[(0, w0)] = xt

        wt_sb = const.tile([P, 17, KB, D], bf16)
        nc.sync.dma_start(wt_sb[:, 0, :, :], ins["wt"][:, 0, :, :])
        nc.sync.dma_start(wt_sb[:, 1, :, :], ins["wt"][:, 1, :, :])
        nc.sync.dma_start(dr_sb[:, sw0_hi:], ins["dr"][:, sw0_hi:])
        for w in range(2, NW):
            nc.sync.dma_start(wt_sb[:, w, :, :], ins["wt"][:, w, :, :])
        # ipt + self weights last: the self transform closes each window's
        # chain, so these are far off the startup critical path
        ipt_sb = const.tile([P, KB, CORE_NODES], bf16)
        nc.sync.dma_start(ipt_sb[:], ins["ipt"][:, :, :])
        nc.sync.dma_start(wt_sb[:, 16, :, :], ins["wt"][:, 16, :, :])

        for swi in range(NSW):
            # --- dense accumulator: out[dest, feat] directly, one PSUM
            # bank per 128-dest block (G/x/cnt blocks as stationary).
            apt = [aps.tile([P, D], f32, tag="ps", name=f"apt{_db}") for _db in range(KB)]

            def _bias():
                for db in range(KB):
                    nc.tensor.matmul(
                        apt[db][:],
                        lhsT=cnt_sb[:, ds(swi * SW + db * P, P)],
                        rhs=baug_sb[:, :],
                        start=True,
                        stop=False,
                    )
            # software pipeline: dense-agg consumption runs DELAY agg
            # groups behind, so the PSUM->SBUF evacuation never stalls the
            # in-order PE.
            DELAY = 2
            dly = DELAY
            gts = {}
            emitted = 0

            def _dense_agg(w):
                gt = gts.pop(w)
                for kb in range(KB):
                    for db in range(KB):
                        nc.tensor.matmul(
                            apt[db][:],
                            lhsT=gt[:, kb, ts(db, P)],
                            rhs=wt_sb[:, w, kb, :],
                            start=False,
                            stop=False,
                        )

            for wi in range(NW):
                if True:
                    w = wi
                    cb, n, wins = bykey[(w, swi)]
                    gpt = [gps.tile([P, SW], f32, tag="gp", name=f"gpt{_mb}") for _mb in range(KB)]
                    xt = xts.pop((swi, w), None)
                    if xt is None:
                        xt = xp.tile([P, NCHMAX * D], bf16, tag="x")
                        # xg streams ride the otherwise-idle Pool queue so
                        # they do not head-of-line block the Act SEQ (which
                        # issues the gt evacuation copies).
                        nc.gpsimd.dma_start(
                            xt[:, : n * D], xg_dram[:, cb * D : (cb + n) * D]
                        )
                    for j in range(n):
                        a, wn = wins[j]
                        at = apl.tile([P, SW], bf16, tag="a")
                        nc.vector.tensor_scalar(
                            at[:, :wn],
                            iota_sb[:, :wn],
                            dr_sb[:, cb + j : cb + j + 1],
                            None,
                            mybir.AluOpType.is_equal,
                        )
                        for mb in range(KB):
                            nc.tensor.matmul(
                                gpt[mb][:, a : a + wn],
                                lhsT=xt[:, j * D + mb * P : j * D + (mb + 1) * P],
                                rhs=at[:, :wn],
                                start=(j == 0),
                                stop=(j == n - 1),
                            )
                    gt = gsb.tile([P, KB, SW], bf16)
                    for mb in range(KB):
                        # PSUM->SBUF evacuation split across Act + DVE so the
                        # first gpt bank frees fast for the next key's agg.
                        if mb % 2 == 0:
                            nc.scalar.activation(
                                gt[:, mb, :], gpt[mb][:],
                                mybir.ActivationFunctionType.Copy,
                            )
                        else:
                            nc.vector.tensor_copy(gt[:, mb, :], gpt[mb][:])
                    gts[w] = gt
                while emitted < NW and wi >= emitted + dly - 1:
                    if emitted == 0:
                        _bias()
                    _dense_agg(emitted)
                    emitted += 1
                    if emitted >= 2:
                        dly = DELAY
            while emitted < NW:
                if emitted == 0:
                    _bias()
                _dense_agg(emitted)
                emitted += 1
            # self transform closes each chain; db-outer so apt[db] chains
            # stop staggered -- each block's relu/store overlaps the
            # remaining self matmuls
            for db in range(KB):
                for kb in range(KB):
                    nc.tensor.matmul(
                        apt[db][:],
                        lhsT=ipt_sb[:, kb, ds(swi * SW + db * P, P)],
                        rhs=wt_sb[:, 16, kb, :],
                        start=False,
                        stop=(kb == KB - 1),
                    )

            # --- relu + store (already [dest, feat])
            for db in range(KB):
                ot = osb.tile([P, D], bf16)
                nc.vector.tensor_scalar(
                    ot[:], apt[db][:], 0.0, None, mybir.AluOpType.max
                )
                nc.sync.dma_start(out_ap[ds(swi * SW + db * P, P), :], ot[:])


def _build(cfg, plan, NCHUNK, NCHMAX):
    nc = bacc.Bacc("TRN2", target_bir_lowering=False, debug=False,
                   num_devices=NCORES)
    f32 = mybir.dt.float32
    ins = {
        "xg": nc.dram_tensor("xg", (P, NCHUNK * D), mybir.dt.bfloat16, kind="ExternalInput").ap(),
        "dr": nc.dram_tensor("dr", (P, NCHUNK), f32, kind="ExternalInput").ap(),
        "iota": nc.dram_tensor("iota", (P, SW), mybir.dt.float16, kind="ExternalInput").ap(),
        "ipt": nc.dram_tensor("ipt", (P, KB, cfg.CORE_NODES), mybir.dt.bfloat16, kind="ExternalInput").ap(),
        "wt": nc.dram_tensor("wt", (P, 17, KB, D), mybir.dt.bfloat16, kind="ExternalInput").ap(),
        "cnt": nc.dram_tensor("cnt", (32, cfg.CORE_NODES), mybir.dt.bfloat16, kind="ExternalInput").ap(),
        "baug": nc.dram_tensor("baug", (32, D), mybir.dt.bfloat16, kind="ExternalInput").ap(),
    }
    out = nc.dram_tensor("out", (cfg.CORE_NODES, D), mybir.dt.bfloat16, kind="ExternalOutput").ap()
    with tile.TileContext(nc) as tc:
        _emit(tc, out, ins, cfg, plan, NCHUNK, NCHMAX)
    nc.compile()
    return nc


def kernel(**inputs):
    global LAST_RESULT
    a = {k: np.asarray(v) for k, v in inputs.items()}
    inp = a["input"].astype(np.float32)
    cfg = make_cfg(inp.shape[0])
    in_maps, plan, NCHUNK, NCHMAX = _host_prep(
        cfg, inp, a["heads"], a["tails"], a["rel"],
        a["W_self"].astype(np.float32), a["b_self"].astype(np.float32),
        a["W_rel"].astype(np.float32), a["b_rel"].astype(np.float32),
    )
    nc = _build(cfg, plan, NCHUNK, NCHMAX)
    res = bass_utils.run_bass_kernel_spmd(
        nc, in_maps, core_ids=list(range(NCORES)),
        trace=os.environ.get("KERNEL_TRACE", "") not in ("", "0"),
    )
    LAST_RESULT = res
    full = np.concatenate([res.results[c]["out"] for c in range(NCORES)], 0)
    return full[: cfg.N].astype(np.float32)
